# revision 28
# baseline (speedup 1.0000x reference)
"""Trainium2 Bass kernel for nn_CVKANTimeSeries (polynomial rewrite).

Reference computation (per batch element b, sequence s, channel d):
  - complex embedding zr/zi = x @ er_w/ei_w + bias, rotated by positional
    phases (cos/sin tables).
  - 4 stacked "polarizing" layers: causal cumulative mean -> magnitude/phase
    -> tiny 1->32->1 (psi_mag) and 2->32->2 (psi_phase) GELU MLPs ->
    residual add of the polarized vector.
  - decode: gelu(zr @ op_w1 + op_b1) @ op_w2 + op_b2.

Key algorithmic observation: both tiny MLPs are *scalar* functions.
  psi_mag:  log_mag_out - log_mag = f_l(log_mag), a fixed smooth 1-D
            function per layer -> fit a degree-8 polynomial P_l(L) of
            L = ln(mag^2) = ln(Sr^2+Si^2) - 2 ln(count) with
            P_l = u + mag_scale*f_l(u), u = ln(exp(L/2)+1e-6).
  psi_phase: acts on the unit vector (cos phi, sin phi); its UN-normalized
            output v(phi) has Fourier content that dies at harmonic 2
            (gelu of ~0.14-amplitude args), so
              v_c(phi) = a0 + a1 cos + b1 sin + a2 cos2 + b2 sin2
            with cos2 = (Sr^2-Si^2)/|S|^2, sin2 = 2 Sr Si/|S|^2.
            The L2 normalization runs on-device through the exp/ln trick:
            r_hat = exp(P_l(L) - 0.5 ln|v|^2), z += r_hat * v.
Counts cancel in the phase (p = Sr/|S|), so invcnt tables disappear.

This removes ALL layer matmuls and ALL Gelu activations (the baseline's
bottleneck: ACT engine 83% busy on 1026 gelu ops + table thrash).  The
only ACT funcs used in layers are Ln/Exp/Square - one table set
(natural_log_exp_and_others), zero table switches.  Per layer:
4 DVE scans, ~26 tensor-tensor/stt ops, 3 tensor-scalar, 9 ACT ops.
Elementwise ops are column-sliced DVE : GPSIMD/Pool ~ 1408 : 640 so both
engines finish together (Pool runs TT at 0.42 efficiency).

Coefficients are fitted at runtime from the actual weights (host-side,
numpy-only) and baked into the module as float immediates; the module
cache is keyed on them.

Sharding: data-parallel over batch (B=8 -> 1 batch element per core).
Per-core layout: channels d (256) as two partition tiles of 128 stored
side by side in the free dim ([128, 2048]); sequence s along free dim.
Embedding and decode stay on the tensor engine with 3-term bf16-split
accumulation (near-fp32).
"""

import math
import os

import ml_dtypes
import numpy as np

import concourse.bacc as bacc
import concourse.bass as bass
import concourse.mybir as mybir
import concourse.tile as tile
from concourse.bass_utils import run_bass_kernel_spmd

F32 = mybir.dt.float32
BF16 = mybir.dt.bfloat16
AF = mybir.ActivationFunctionType
ALU = mybir.AluOpType
NPBF = ml_dtypes.bfloat16

B, S, D, H, IN, L = 8, 1024, 256, 32, 64, 4
NCORES = 8
T = 2               # d-tiles of 128 partitions
NBLK = 2            # 512-column blocks for embedding/decode matmuls
FREE = T * S        # 2048 columns: the two d-tiles side by side
SPL = int(os.environ.get("KERNEL_SPL", "512"))  # DVE columns per 2048 of TT ops
HALVES = bool(int(os.environ.get("KERNEL_HALVES", "1")))
# TSCONV: emit each scalar_tensor_tensor as tensor_scalar (DVE, 2x mode) +
# tensor_tensor (DVE/Pool split) instead of one 1x-rate DVE stt.  Trades
# DVE cycles for Pool cycles; worthwhile if Pool TT is fast on real HW.
TSCONV = bool(int(os.environ.get("KERNEL_TSCONV", "0")))
NM = 8              # magnitude polynomial degree
LLO, LHI = -26.0, 10.0   # fit range for L = ln(mag^2)
EPS_MAG = 1e-6

_BUILT = {}         # (coeffs, reps) -> Bass module
LAST_RESULT = None  # BassKernelResults of the most recent run (for profiling)
LAST_COEFFS = None  # set by _prep_consts; used by _get_built for test.py


def _sliced(fv, fp, *aps, whole=False):
    """Emit an elementwise op column-sliced across DVE (fv) and Pool (fp)."""
    if whole:
        fv(*[a for a in aps])
        return
    fv(*[a[:, :SPL] for a in aps])
    fp(*[a[:, SPL:] for a in aps])


def _build_module(coeffs, reps=1):
    """Emit the Bass/Tile IR. `coeffs` carries all weight-derived immediates:
    (alpha, ((mono...), (au...), (av...)) x L, op_b2)."""
    alpha, per_layer, op_b2 = coeffs
    nc = bacc.Bacc("TRN2", debug=False, num_devices=NCORES)

    dram = {}

    def din(name, shape, dt=F32):
        dram[name] = nc.dram_tensor(name, shape, dt, kind="ExternalInput")
        return dram[name]

    din("xaug_h", [IN + 1, S], BF16)
    din("xaug_l", [IN + 1, S], BF16)
    din("c_embw_rh", [IN + 1, D], BF16)
    din("c_embw_rl", [IN + 1, D], BF16)
    din("c_embw_ih", [IN + 1, D], BF16)
    din("c_embw_il", [IN + 1, D], BF16)
    din("c_rot_c", [128, FREE])
    din("c_rot_s", [128, FREE])
    din("c_lncnt2p", [128, FREE])
    din("c_scal", [1, 8])  # per-layer exp bias mono[0], op_b2
    din("c_dec1h", [128, T * H], BF16)
    din("c_dec1l", [128, T * H], BF16)
    din("c_dec2h", [H, 1], BF16)
    din("c_dec2l", [H, 1], BF16)
    din("c_decb1", [H, 1])
    out_dram = nc.dram_tensor("out", [1, S], F32, kind="ExternalOutput")

    with tile.TileContext(nc) as tc:
        # Pre-load the combined Ln/Exp/Square ACT table set: the automatic
        # pass would otherwise thrash natural_log <-> exp_and_others on
        # every Ln->Exp transition (4 loads x ~1.3us per layer).
        from concourse.hw_specs import get_activation_tables
        tabs = list(get_activation_tables(nc.m.arch).items())
        cid = [i for i, (nm, _) in enumerate(tabs)
               if nm == "natural_log_exp_and_others"]
        if cid:
            ld = mybir.InstLoadActFuncSet(
                name=nc.get_next_instruction_name(), act_func_set_id=cid[0])
            ld.engine = mybir.EngineType.Activation
            nc.scalar.add_instruction(ld)
        with tc.tile_pool(name="persist", bufs=1) as persist:
            # ---- persistent constants ----
            lncnt2p = persist.tile([128, FREE], F32)
            nc.sync.dma_start(out=lncnt2p, in_=dram["c_lncnt2p"].ap())
            rot_c = persist.tile([128, FREE], F32)
            nc.sync.dma_start(out=rot_c, in_=dram["c_rot_c"].ap())
            rot_s = persist.tile([128, FREE], F32)
            nc.sync.dma_start(out=rot_s, in_=dram["c_rot_s"].ap())
            dec1h = persist.tile([128, T * H], BF16)
            nc.sync.dma_start(out=dec1h, in_=dram["c_dec1h"].ap())
            dec1l = persist.tile([128, T * H], BF16)
            nc.sync.dma_start(out=dec1l, in_=dram["c_dec1l"].ap())
            dec2h = persist.tile([H, 1], BF16)
            nc.sync.dma_start(out=dec2h, in_=dram["c_dec2h"].ap())
            dec2l = persist.tile([H, 1], BF16)
            nc.sync.dma_start(out=dec2l, in_=dram["c_dec2l"].ap())
            decb1 = persist.tile([H, 1], F32)
            nc.sync.dma_start(out=decb1, in_=dram["c_decb1"].ap())
            xh = persist.tile([IN + 1, S], BF16)
            nc.sync.dma_start(out=xh, in_=dram["xaug_h"].ap())
            xl = persist.tile([IN + 1, S], BF16)
            nc.sync.dma_start(out=xl, in_=dram["xaug_l"].ap())
            ewrh = persist.tile([IN + 1, D], BF16)
            nc.sync.dma_start(out=ewrh, in_=dram["c_embw_rh"].ap())
            ewrl = persist.tile([IN + 1, D], BF16)
            nc.sync.dma_start(out=ewrl, in_=dram["c_embw_rl"].ap())
            ewih = persist.tile([IN + 1, D], BF16)
            nc.sync.dma_start(out=ewih, in_=dram["c_embw_ih"].ap())
            ewil = persist.tile([IN + 1, D], BF16)
            nc.sync.dma_start(out=ewil, in_=dram["c_embw_il"].ap())

            # broadcast row of c_scal to 128 partitions for bias APs
            scal_b = persist.tile([128, 8], F32)
            nc.sync.dma_start(
                out=scal_b,
                in_=bass.AP(
                    tensor=dram["c_scal"].ap().tensor,
                    offset=dram["c_scal"].ap().offset,
                    ap=[[0, 128], [1, 8]],
                ),
            )

            # ---- state ----
            zr = persist.tile([128, FREE], F32, name="zr")
            zi = persist.tile([128, FREE], F32, name="zi")

            with tc.tile_pool(name="work", bufs=1) as work, \
                 tc.tile_pool(name="psh", bufs=1, space="PSUM") as psh:
                for _rep in range(reps):
                    _emit_body(
                        nc, tc, dram, out_dram, alpha, per_layer, scal_b,
                        lncnt2p, rot_c, rot_s,
                        dec1h, dec1l, dec2h, dec2l, decb1,
                        xh, xl, ewrh, ewrl, ewih, ewil,
                        zr, zi, work, psh,
                    )

    nc.compile()
    return nc


def _emit_body(nc, tc, dram, out_dram, alpha, per_layer, scal_b,
               lncnt2p, rot_c, rot_s,
               dec1h, dec1l, dec2h, dec2l, decb1,
               xh, xl, ewrh, ewrl, ewih, ewil,
               zr, zi, work, psh):
    # The real ISA runs tensor_scalar/scalar_tensor_tensor/scans ONLY on DVE
    # (GPSIMD has tensor_tensor ucode but no TensorScalarPtr support), so:
    #   - plain TT ops: column-split DVE:Pool = SPL:(FREE-SPL)
    #   - stt / tensor_scalar / scans: DVE
    # With HALVES the two d-tiles are emitted as separate [*,1024] pieces so
    # tile-0's chain can run while tile-1 is still scanning.
    if HALVES:
        sph = SPL // 2
        tt_pieces = [(slice(0, sph), slice(sph, S)),
                     (slice(S, S + sph), slice(S + sph, FREE))]
        cols = [slice(0, S), slice(S, FREE)]
    else:
        tt_pieces = [(slice(0, SPL), slice(SPL, FREE))]
        cols = [slice(0, FREE)]

    def v_tt(out, in0, in1, op):
        for dv, pl in tt_pieces:
            if dv.stop > dv.start:
                nc.vector.tensor_tensor(out=out[:, dv], in0=in0[:, dv],
                                        in1=in1[:, dv], op=op)
            nc.gpsimd.tensor_tensor(out=out[:, pl], in0=in0[:, pl],
                                    in1=in1[:, pl], op=op)

    def v_stt(out, in0, scalar, in1, op0, op1):
        for cs in cols:
            nc.vector.scalar_tensor_tensor(
                out=out[:, cs], in0=in0[:, cs], scalar=scalar,
                in1=in1[:, cs], op0=op0, op1=op1)

    def v_ts(out, in0, s1, s2, op0, op1=None):
        for cs in cols:
            if op1 is None:
                nc.vector.tensor_scalar(out=out[:, cs], in0=in0[:, cs],
                                        scalar1=s1, scalar2=None, op0=op0)
            else:
                nc.vector.tensor_scalar(out=out[:, cs], in0=in0[:, cs],
                                        scalar1=s1, scalar2=s2, op0=op0, op1=op1)

    def v_act(out, in_, func, **kw):
        for cs in cols:
            nc.scalar.activation(out[:, cs], in_[:, cs], func, **kw)

    # ---- embedding + rotation (3-term bf16-split matmuls) ----
    for t in range(T):
        dcol = slice(128 * t, 128 * t + 128)
        for n in range(NBLK):
            cs = slice(512 * n, 512 * n + 512)
            tcs = slice(S * t + 512 * n, S * t + 512 * n + 512)
            ps_er = psh.tile([128, 512], F32, tag="pser", bufs=2, name="ps_er")
            ps_ei = psh.tile([128, 512], F32, tag="psei", bufs=2, name="ps_ei")
            for ps, wh, wl in ((ps_er, ewrh, ewrl), (ps_ei, ewih, ewil)):
                nc.tensor.matmul(ps, wh[:, dcol], xh[:, cs],
                                 start=True, stop=False)
                nc.tensor.matmul(ps, wh[:, dcol], xl[:, cs],
                                 start=False, stop=False)
                nc.tensor.matmul(ps, wl[:, dcol], xh[:, cs],
                                 start=False, stop=True)
            t1 = work.tile([128, 512], F32, tag="embt1", bufs=2, name="t1")
            t2 = work.tile([128, 512], F32, tag="embt2", bufs=2, name="t2")
            t1b = work.tile([128, 512], F32, tag="embt1b", bufs=2, name="t1b")
            t2b = work.tile([128, 512], F32, tag="embt2b", bufs=2, name="t2b")
            # rotation: zr = er*c - ei*s ; zi = er*s + ei*c
            # (GPSIMD cannot read PSUM, so the ps_* reads stay on DVE;
            # the PSUM-free combines go to Pool.)
            nc.vector.tensor_tensor(out=t1, in0=ps_er, in1=rot_c[:, tcs], op=ALU.mult)
            nc.vector.tensor_tensor(out=t2, in0=ps_ei, in1=rot_s[:, tcs], op=ALU.mult)
            nc.gpsimd.tensor_tensor(out=zr[:, tcs], in0=t1, in1=t2, op=ALU.subtract)
            nc.vector.tensor_tensor(out=t1b, in0=ps_er, in1=rot_s[:, tcs], op=ALU.mult)
            nc.vector.tensor_tensor(out=t2b, in0=ps_ei, in1=rot_c[:, tcs], op=ALU.mult)
            nc.gpsimd.tensor_tensor(out=zi[:, tcs], in0=t1b, in1=t2b, op=ALU.add)

    # ---- layers (no matmuls, no gelu; Ln/Exp/Square only) ----
    # Residual updates are scan-fused: layer l produces the residual pair
    # (ur, ui) = rh*(vu, vv); layer l+1's cumsum consumes them via the
    # scan's second data operand (state = (zr + state) + ur), so the
    # z += u materialization happens OFF the critical path during l+1.
    ur = ui = None
    for l in range(L):
        mono, cu, cv = per_layer[l]
        Sr = work.tile([128, FREE], F32, tag="Sr", name="Sr")
        Si = work.tile([128, FREE], F32, tag="Si", name="Si")
        # causal cumsums: independent per 1024-tile; zr halves on DVE,
        # zi halves on Pool so the scans overlap.
        for t in range(T):
            h = slice(S * t, S * t + S)
            if ur is None:
                nc.vector.tensor_tensor_scan(
                    out=Sr[:, h], data0=zr[:, h], data1=zr[:, h],
                    initial=0.0, op0=ALU.add, op1=ALU.bypass)
                nc.vector.tensor_tensor_scan(
                    out=Si[:, h], data0=zi[:, h], data1=zi[:, h],
                    initial=0.0, op0=ALU.add, op1=ALU.bypass)
            else:
                nc.vector.tensor_tensor_scan(
                    out=Sr[:, h], data0=zr[:, h], data1=ur[:, h],
                    initial=0.0, op0=ALU.add, op1=ALU.add)
                nc.vector.tensor_tensor_scan(
                    out=Si[:, h], data0=zi[:, h], data1=ui[:, h],
                    initial=0.0, op0=ALU.add, op1=ALU.add)
        if ur is not None:
            # fold the previous layer's residual into the state tensors
            # (consumers: this layer's magnitude/phase chain is already fed
            # by the fused scans; only the NEXT layer's scan reads zr/zi,
            # and after the last layer only zr is read, by decode).
            v_tt(zr, zr, ur, ALU.add)
            if l < L - 1:
                v_tt(zi, zi, ui, ALU.add)

        sqr = work.tile([128, FREE], F32, tag="sqr", name="sqr")
        sqi = work.tile([128, FREE], F32, tag="sqi", name="sqi")
        v_act(sqr, Sr, AF.Square)
        v_act(sqi, Si, AF.Square)
        d2 = work.tile([128, FREE], F32, tag="d2", name="d2")
        v_tt(d2, sqr, sqi, ALU.subtract)
        s2 = sqr  # in-place: sqr dead after d2/s2
        v_tt(s2, sqr, sqi, ALU.add)
        m = work.tile([128, FREE], F32, tag="m", name="m")
        v_tt(m, Sr, Si, ALU.mult)
        lam = sqi  # in-place: sqi dead after s2
        v_act(lam, s2, AF.Ln)
        inv = work.tile([128, FREE], F32, tag="inv", name="inv")
        v_act(inv, lam, AF.Exp, scale=-0.5)
        inv2 = work.tile([128, FREE], F32, tag="inv2", name="inv2")
        v_act(inv2, lam, AF.Exp, scale=-1.0)
        # t-variable for the magnitude polynomial: tv = alpha*lam - lncnt2p
        tv = lam  # in-place: lam dead after inv/inv2
        v_stt(tv, lam, float(alpha), lncnt2p, ALU.mult, ALU.subtract)
        # harmonic basis: p=cos, q=sin, c2=cos2, s2p=sin2 (sans factor 2)
        p = Sr  # in-place: Sr dead after sqr/m
        q = Si
        v_tt(p, Sr, inv, ALU.mult)
        v_tt(q, Si, inv, ALU.mult)
        c2 = d2
        v_tt(c2, d2, inv2, ALU.mult)
        s2p = m
        v_tt(s2p, m, inv2, ALU.mult)
        # magnitude polynomial P = sum_{k>=1} mono[k] t^k (Horner via stt);
        # mono[0] goes into the final Exp bias.
        P = work.tile([128, FREE], F32, tag="P", name="P")
        v_ts(P, tv, float(mono[NM]), None, ALU.mult)
        for k in range(NM - 1, 0, -1):
            v_stt(P, P, float(mono[k]), tv, ALU.add, ALU.mult)
        # phase assembly: v_c = a0 + a1*p + b1*q + a2*c2 + (2*b2)*s2p
        vu = work.tile([128, FREE], F32, tag="vu", name="vu")
        vv = work.tile([128, FREE], F32, tag="vv", name="vv")
        for vt, (a0, a1, b1, a2, b22) in ((vu, cu), (vv, cv)):
            v_ts(vt, s2p, float(b22), float(a0), ALU.mult, ALU.add)
            v_stt(vt, c2, float(a2), vt, ALU.mult, ALU.add)
            v_stt(vt, q, float(b1), vt, ALU.mult, ALU.add)
            v_stt(vt, p, float(a1), vt, ALU.mult, ALU.add)
        # normalization + residual: r_hat = exp(P + mono0 - 0.5 ln|v|^2)
        svu = work.tile([128, FREE], F32, tag="svu", name="svu")
        svv = work.tile([128, FREE], F32, tag="svv", name="svv")
        v_act(svu, vu, AF.Square)
        v_act(svv, vv, AF.Square)
        n2 = svu
        v_tt(n2, svu, svv, ALU.add)
        lam2 = svv
        v_act(lam2, n2, AF.Ln)
        E = n2
        v_stt(E, lam2, -0.5, P, ALU.mult, ALU.add)
        rh = P  # in-place: P dead after E
        v_act(rh, E, AF.Exp, bias=scal_b[:, l:l + 1])
        # residual pair for the next layer's fused scan (folded there)
        ur = work.tile([128, FREE], F32, tag="ur", name="ur")
        ui = work.tile([128, FREE], F32, tag="ui", name="ui")
        v_tt(ur, rh, vu, ALU.mult)
        v_tt(ui, rh, vv, ALU.mult)

    # fold the last layer's residual (decode only needs zr)
    v_tt(zr, zr, ur, ALU.add)

    # ---- decode (3-term bf16 splits) ----
    zrh = work.tile([128, FREE], BF16, tag="zrh", name="zrh")
    zrl = work.tile([128, FREE], BF16, tag="zrl", name="zrl")
    nc.vector.tensor_copy(out=zrh, in_=zr)
    v_tt(zrl, zr, zrh, ALU.subtract)
    hd = work.tile([H, S], F32, tag="hd", name="hd")
    for n in range(NBLK):
        cs = slice(512 * n, 512 * n + 512)
        ps_dec = psh.tile([H, 512], F32, tag="pser", bufs=2, name="ps_dec")
        for t in range(T):
            hcol = slice(H * t, H * t + H)
            tcs = slice(S * t + 512 * n, S * t + 512 * n + 512)
            nc.tensor.matmul(ps_dec, dec1h[:, hcol], zrh[:, tcs],
                             start=(t == 0), stop=False)
            nc.tensor.matmul(ps_dec, dec1h[:, hcol], zrl[:, tcs],
                             start=False, stop=False)
            nc.tensor.matmul(ps_dec, dec1l[:, hcol], zrh[:, tcs],
                             start=False, stop=(t == T - 1))
        nc.scalar.activation(hd[:, cs], ps_dec, AF.Gelu, bias=decb1)
    hdh = work.tile([H, S], BF16, tag="hdh", name="hdh")
    hdl = work.tile([H, S], BF16, tag="hdl", name="hdl")
    nc.vector.tensor_copy(out=hdh, in_=hd)
    nc.vector.tensor_tensor(out=hdl, in0=hd, in1=hdh, op=ALU.subtract)
    preds = work.tile([1, S], F32, tag="preds", name="preds")
    for n in range(NBLK):
        cs = slice(512 * n, 512 * n + 512)
        ps_out = psh.tile([1, 512], F32, tag="psei", bufs=2, name="ps_out")
        nc.tensor.matmul(ps_out, dec2h, hdh[:, cs], start=True, stop=False)
        nc.tensor.matmul(ps_out, dec2h, hdl[:, cs], start=False, stop=False)
        nc.tensor.matmul(ps_out, dec2l, hdh[:, cs], start=False, stop=True)
        nc.scalar.activation(preds[:, cs], ps_out, AF.Identity,
                             bias=scal_b[0:1, 4:5])
    nc.sync.dma_start(out=out_dram.ap(), in_=preds)


def _split_bf16(a):
    hi = a.astype(NPBF)
    lo = (a - hi.astype(np.float32)).astype(NPBF)
    return hi, lo


_erf = np.vectorize(math.erf)


def _gelu_np(x):
    return 0.5 * x * (1.0 + _erf(x / np.sqrt(2.0)))


def _fit_coeffs(inputs):
    """Fit per-layer polynomial/harmonic coefficients (float64 host math)."""
    f64 = np.float64
    pm_w1 = np.asarray(inputs["pm_w1"], f64)
    pm_b1 = np.asarray(inputs["pm_b1"], f64)
    pm_w2 = np.asarray(inputs["pm_w2"], f64)
    pm_b2 = np.asarray(inputs["pm_b2"], f64)
    pp_w1 = np.asarray(inputs["pp_w1"], f64)
    pp_b1 = np.asarray(inputs["pp_b1"], f64)
    pp_w2 = np.asarray(inputs["pp_w2"], f64)
    pp_b2 = np.asarray(inputs["pp_b2"], f64)
    mag_scale = np.asarray(inputs["mag_scale"], f64)

    alpha = 2.0 / (LHI - LLO)
    per_layer = []
    Lg = np.linspace(LLO, LHI, 4001)
    tg = (2 * Lg - (LHI + LLO)) / (LHI - LLO)
    NG = 4096
    phi = np.linspace(0, 2 * np.pi, NG, endpoint=False)
    pv = np.stack([np.cos(phi), np.sin(phi)], axis=-1)
    basis = np.stack([np.ones(NG), np.cos(phi), np.sin(phi),
                      np.cos(2 * phi), np.sin(2 * phi)], axis=-1)
    for l in range(L):
        # magnitude path: P(L) = u + mag_scale*f(u), u = ln(exp(L/2)+eps)
        u = np.log(np.exp(0.5 * Lg) + EPS_MAG)
        h = _gelu_np(u[:, None] * pm_w1[l][0] + pm_b1[l])
        target = u + mag_scale[l] * (h @ pm_w2[l][:, 0] + pm_b2[l][0])
        cf = np.polynomial.chebyshev.chebfit(tg, target, NM)
        mono = np.polynomial.chebyshev.cheb2poly(cf)
        # phase path: direction-weighted harmonic fit of the raw MLP output
        v = _gelu_np(pv @ pp_w1[l] + pp_b1[l]) @ pp_w2[l] + pp_b2[l]
        w = 1.0 / np.linalg.norm(v, axis=-1)
        comp = []
        for c in range(2):
            sol, *_ = np.linalg.lstsq(basis * w[:, None], v[:, c] * w,
                                      rcond=None)
            a0, a1, b1, a2, b2 = sol
            comp.append((float(a0), float(a1), float(b1), float(a2),
                         float(2.0 * b2)))
        per_layer.append((tuple(float(x) for x in mono),
                          tuple(comp[0]), tuple(comp[1])))
    op_b2 = float(np.asarray(inputs["op_b2"], f64)[0])
    return (float(alpha), tuple(per_layer), op_b2)


def _prep_consts(inputs):
    """Build weight-derived DRAM constant arrays + baked coefficients."""
    global LAST_COEFFS
    f32 = np.float32
    er_w = np.asarray(inputs["er_w"], f32)
    er_b = np.asarray(inputs["er_b"], f32)
    ei_w = np.asarray(inputs["ei_w"], f32)
    ei_b = np.asarray(inputs["ei_b"], f32)
    op_w1 = np.asarray(inputs["op_w1"], f32)
    op_b1 = np.asarray(inputs["op_b1"], f32)
    op_w2 = np.asarray(inputs["op_w2"], f32)

    c = {}
    embr = np.concatenate([er_w, er_b[None, :]], axis=0)
    embi = np.concatenate([ei_w, ei_b[None, :]], axis=0)
    c["c_embw_rh"], c["c_embw_rl"] = _split_bf16(embr)
    c["c_embw_ih"], c["c_embw_il"] = _split_bf16(embi)

    pos = np.arange(S, dtype=f32)[:, None]
    freq = np.exp(-np.log(10000.0) * np.arange(D, dtype=f32) / D).astype(f32)
    theta = (pos * freq[None, :]).astype(f32)  # [S, D]
    rc = np.cos(theta).astype(f32)
    rs = np.sin(theta).astype(f32)
    rot_c = np.empty((128, FREE), f32)
    rot_s = np.empty((128, FREE), f32)
    for t in range(T):
        rot_c[:, S * t:S * t + S] = rc[:, 128 * t:128 * t + 128].T
        rot_s[:, S * t:S * t + S] = rs[:, 128 * t:128 * t + 128].T
    c["c_rot_c"] = rot_c
    c["c_rot_s"] = rot_s

    # tv = alpha*Lambda - lncnt2p with lncnt2p = alpha*(2 ln(count) + mid)
    mid = (LHI + LLO) / 2.0
    alpha = 2.0 / (LHI - LLO)
    ln2c = 2.0 * np.log(np.arange(1, S + 1, dtype=np.float64))
    row = (alpha * (ln2c + mid)).astype(f32)[None, :]
    c["c_lncnt2p"] = np.broadcast_to(
        np.concatenate([row, row], axis=1), (128, FREE)).copy()

    dec1 = np.zeros((128, T * H), f32)
    for t in range(T):
        dec1[:, H * t:H * t + H] = op_w1[128 * t:128 * t + 128, :]
    c["c_dec1h"], c["c_dec1l"] = _split_bf16(dec1)
    c["c_dec2h"], c["c_dec2l"] = _split_bf16(op_w2.astype(f32))
    c["c_decb1"] = op_b1[:, None].astype(f32)

    LAST_COEFFS = _fit_coeffs(inputs)
    _, per_layer, op_b2 = LAST_COEFFS
    scal = np.zeros((1, 8), f32)
    for l in range(L):
        scal[0, l] = per_layer[l][0][0]   # mono[0] -> exp bias
    scal[0, 4] = op_b2
    c["c_scal"] = scal
    return c


def _get_built(reps=1, coeffs=None):
    if coeffs is None:
        coeffs = LAST_COEFFS
    assert coeffs is not None, "call _prep_consts/_make_in_maps first"
    key = (coeffs, reps, SPL, HALVES)
    if key not in _BUILT:
        _BUILT[key] = _build_module(coeffs, reps)
    return _BUILT[key]


def _make_in_maps(inputs):
    consts = _prep_consts(inputs)
    x = np.asarray(inputs["x"], np.float32)  # [B, S, IN]
    in_maps = []
    for b in range(NCORES):
        mday = dict(consts)
        xaug = np.empty((IN + 1, S), np.float32)
        xaug[:IN, :] = x[b].T
        xaug[IN, :] = 1.0
        mday["xaug_h"], mday["xaug_l"] = _split_bf16(xaug)
        in_maps.append(mday)
    return in_maps


def kernel(**inputs):
    in_maps = _make_in_maps(inputs)
    nc = _get_built()

    global LAST_RESULT
    trace = bool(int(os.environ.get("KERNEL_TRACE", "0")))
    res = run_bass_kernel_spmd(
        nc, in_maps, core_ids=list(range(NCORES)), trace=trace,
    )
    LAST_RESULT = res

    out = np.empty((B, S, 1), np.float32)
    for b in range(NCORES):
        out[b, :, 0] = res.results[b]["out"][0]
    return out
